# revision 48
# baseline (speedup 1.0000x reference)
"""MoE FFN (8 experts, top-2) on 8 Trainium2 NeuronCores.

Strategy: expert parallelism with host-side token routing, load-balanced
by splitting each expert's FFN along the intermediate dim.
  - Host computes the (tiny) gate: logits = x @ gate_w.T, top-2, softmax.
  - Tokens are gathered per expert and the expert FFN is split into two
    half-intermediate units (e, half).  The 16 units are sorted by token
    count; each core runs one of the 8 heaviest (padded to C_A) plus one
    of the 8 lightest (padded to C_B), so per-core work is
    (C_A + C_B) / 2 ~ 2112 token-equivalents instead of max_e C_e = 2176.
  - Host sums each expert's two half partials, adds b2, and scatters with
    the combine weights.

Device kernel layout (per core, per unit, per <=512-token tile):
  FFN1: psum[i128, tok] += W1h[k*128:, m*128:].T @ xT[k*128:, tok]
        h = gelu(psum + b1h)          (ACT, writes bf16)
  FFN2: psum[h128, tok] += W2h[k*128:, m*128:].T @ h[k*128:, tok]
        partial_y = psum              (DVE copy, f32; b2 added on host)
Weights resident in SBUF as bf16 (two 8 MB unit sets).  DMA discipline:
dma_start costs its issuing engine ~600ns, so transfers are batched into
wide-AP chunks; the SP queue carries weights + y, the Act queue carries
only biases, the first x tile, and mid-tile x prefetches.
"""

import sys
import types

import numpy as np
import ml_dtypes

import concourse.bass as bass
import concourse.tile as tile
from concourse import mybir
from concourse.bass_utils import run_bass_kernel_spmd
from bass_rust import ScopedClock, VectorClock


def _ensure_axon_hooks():
    """run_bass_kernel_spmd(trace=True) under axon imports antenv.axon_hooks,
    which this image's antenv lacks.  Register an equivalent module backed by
    trn_agent_boot's ctypes NTFF hook so tracing works (and trace=False paths
    are unaffected)."""
    try:
        import antenv.axon_hooks  # noqa: F401
        return
    except ImportError:
        pass
    hook = None
    try:
        from trn_agent_boot.trn_boot import _ntff_profile_via_ctypes
        hook = _ntff_profile_via_ctypes("/opt/axon/libaxon_pjrt.so")
    except Exception:
        hook = None
    mod = types.ModuleType("antenv.axon_hooks")
    _state = {"hook": hook}
    mod.get_axon_ntff_profile_hook = lambda: _state["hook"]
    mod.set_axon_ntff_profile_hook = lambda h: _state.__setitem__("hook", h)
    sys.modules["antenv.axon_hooks"] = mod
    try:
        import antenv
        antenv.axon_hooks = mod
    except ImportError:
        pass


_ensure_axon_hooks()

H = 1024          # hidden
I = 4096          # intermediate
E = 8             # experts
NCORES = 8
BF16 = mybir.dt.bfloat16
F32 = mybir.dt.float32


class _TC(tile.TileContext):
    """TileContext whose tail drain splits its sem waits across SP nops.

    The walrus pinned in this container rejects a Drain instruction carrying
    more than a couple of sync waits ("Too many sync wait commands",
    CoreV3GenImpl.cpp:104).  Emit one wait-carrier nop per logical processor
    instead, then a waitless drain.
    """

    def _drain_and_barrier(self, tick_clock, wait_clock):
        nc = self.nc
        gc = tick_clock.global_clock
        ticks = eval(repr(gc).replace("VectorClock(", "").rstrip(")"))
        for i, t in enumerate(ticks):
            if t > 0:
                partial = [0] * len(ticks)
                partial[i] = t
                carrier = nc.sync.nop(nofuse=True, hint=f"drain_wait_{i}")
                wait_clock.add_sem_waits(
                    carrier.ins, ScopedClock({None: VectorClock(partial)})
                )
        nc.sync.drain()
        nc.all_engine_barrier()
        assert self.sems is not None
        popped = nc._tile_sem_poison_stack.pop()
        assert popped is self._sem_poison
        # The ~60 serialized EVENT_SEMAPHORE clears cost ~8us of tail.
        # Each kernel() invocation compiles and executes a fresh NEFF
        # exactly once, so end-state semaphore values are never observed;
        # free the handles bass-side without emitting clear instructions.
        for s in self.sems.allocated().values():
            nc.release_semaphore(s)
        nc.all_engine_barrier()


def _split_waits(nc, maxw=1):
    """The pinned walrus rejects instructions carrying more than one
    embedded sync wait ("Too many sync wait commands").  Hoist excess waits
    onto freshly inserted same-engine nops placed directly before the
    instruction — the engine sequencer executes them in order, so the
    semantics are identical."""
    for fn in nc.m.functions:
        for bb in fn.blocks:
            new = []
            changed = False
            for inst in bb.instructions:
                si = inst.sync_info
                waits = list(si.on_wait) if si is not None else []
                if len(waits) > maxw:
                    changed = True
                    n_extra = len(waits) - maxw
                    for i in range(0, n_extra, maxw):
                        nop = mybir.InstNoOp(
                            name=nc.get_next_instruction_name(),
                            engine=inst.engine,
                            sync_info=mybir.SyncInfo(
                                on_wait=waits[i:i + maxw], on_update=[]
                            ),
                            bass_nofuse=True,
                        )
                        nc.register_instruction(nop, overwrite=True)
                        new.append(nop)
                    si.on_wait = waits[n_extra:]
                new.append(inst)
            if changed:
                bb.instructions = new


def _token_tiles(C):
    # Remainder tile last: the first (full) tile's FFN1 masks the W2 load.
    tiles = [512] * (C // 512)
    if C % 512:
        tiles.append(C % 512)
    return tiles


I2 = I // 2       # intermediate half per unit
KH = H // 128     # 8  k-tiles over hidden
KI2 = I2 // 128   # 16 k/m-tiles over the intermediate half


def _build(Cs):
    """Two half-expert FFN units per core (load rebalance).

    Each unit u computes, over C_u tokens of one expert, the FFN restricted
    to one half of the intermediate dim:  partial_y = gelu(x W1h.T + b1h)
    @ W2h.T  (b2 is added on the host when the two halves are combined).
    Splitting along I keeps per-unit weights at 8 MB, so a core holds two
    units (16 MB) and the 8 largest token loads pair with the 8 smallest:
    per-core work drops from max_e C_e to (C_A + C_B) / 2.
    """
    nc = bass.Bass()
    xts, w1ts, w2ts, b1ts, yts = [], [], [], [], []
    for u, C in enumerate(Cs):
        s = "ab"[u]
        xts.append(nc.declare_dram_parameter(f"x{s}", [H, C], BF16,
                                             isOutput=False))
        w1ts.append(nc.declare_dram_parameter(f"w1{s}", [H, I2], BF16,
                                              isOutput=False))
        w2ts.append(nc.declare_dram_parameter(f"w2{s}", [I2, H], BF16,
                                              isOutput=False))
        b1ts.append(nc.declare_dram_parameter(f"b1{s}", [128, KI2], F32,
                                              isOutput=False))
        yts.append(nc.declare_dram_parameter(f"y{s}", [H, C], BF16,
                                             isOutput=True))

    # Flat tile schedule across both units: (unit, tw, off)
    sched = []
    for u, C in enumerate(Cs):
        off = 0
        for tw in _token_tiles(C):
            sched.append((u, tw, off))
            off += tw

    with _TC(nc) as tc:
        with (
            tc.tile_pool(name="weights", bufs=1) as wpool,
            tc.tile_pool(name="bias", bufs=1) as bpool,
            tc.tile_pool(name="x", bufs=3) as xpool,
            tc.tile_pool(name="h", bufs=1) as hpool,
            tc.tile_pool(name="o", bufs=4) as opool,
            tc.tile_pool(name="ps1", bufs=4, space="PSUM") as ps1pool,
            tc.tile_pool(name="ps2", bufs=4, space="PSUM") as ps2pool,
        ):
            # Every dma_start costs its issuing ENGINE ~600ns of dispatch
            # time, so transfers are batched into wide-AP DMAs and almost
            # all dispatch lands on the otherwise-idle SP queue.  The Act
            # queue carries only the latency-critical startup set (biases +
            # first x tile) and the per-tile x prefetch.
            xss = {}

            def emit_x(si, eng, nchunk=1):
                u, tw, off = sched[si]
                xs = xpool.tile([128, KH, tw], BF16, tag="xt",
                                name=f"xs_{si}")
                kstep = KH // nchunk
                for k0 in range(0, KH, kstep):
                    eng.dma_start(
                        xs[:, k0:k0 + kstep, :],
                        xts[u][k0 * 128:(k0 + kstep) * 128, off:off + tw]
                        .rearrange("(k p) c -> p k c", p=128),
                    )
                xss[si] = xs

            # Startup-critical transfers in several chunks so the HWDGE
            # spreads them across channels (one big strided DMA runs
            # ~100 GB/s on a single channel).
            emit_x(0, nc.scalar, nchunk=4)

            # Biases after x0: they gate only the first gelu (~2us after
            # MM#0), while x0 gates MM#0 itself - so x0 dispatches first.
            b1ss = []
            for u in range(2):
                b1s = bpool.tile([128, KI2], F32, tag=f"b1{u}",
                                 name=f"b1s{u}")
                nc.scalar.dma_start(b1s[:], b1ts[u][:])
                b1ss.append(b1s)

            # Unit-a W1 in column phases on SP (small first phase unblocks
            # the first psum groups); then W2a, then unit-b weights coarse
            # (consumed only ~230us in).
            w1ss, w2ss = [], []
            for u in range(2):
                w1ss.append(wpool.tile([128, KH, I2], BF16, tag=f"w1{u}",
                                       name=f"w1s{u}"))
                w2ss.append(wpool.tile([128, KI2, H], BF16, tag=f"w2{u}",
                                       name=f"w2s{u}"))

            def w1_phase(u, lo, hi, ksplit=1):
                kstep = KH // ksplit
                for k0 in range(0, KH, kstep):
                    nc.sync.dma_start(
                        w1ss[u][:, k0:k0 + kstep, lo:hi],
                        w1ts[u][k0 * 128:(k0 + kstep) * 128, lo:hi]
                        .rearrange("(k p) c -> p k c", p=128),
                    )

            def w2_load(u, ksplit=2):
                kstep = KI2 // ksplit
                for k0 in range(0, KI2, kstep):
                    nc.sync.dma_start(
                        w2ss[u][:, k0:k0 + kstep, :],
                        w2ts[u][k0 * 128:(k0 + kstep) * 128, :]
                        .rearrange("(k p) c -> p k c", p=128),
                    )

            w1_phase(0, 0, 256, ksplit=4)
            for lo, hi in [(256, 512), (512, 1024), (1024, 1536),
                           (1536, I2)]:
                w1_phase(0, lo, hi, ksplit=2)
            w2_load(0)
            w1_phase(1, 0, I2, ksplit=2)
            w2_load(1)

            for si, (u, tw, off) in enumerate(sched):
                xs = xss[si]
                w1s, w2s, b1s, yt = w1ss[u], w2ss[u], b1ss[u], yts[u]
                ht = hpool.tile([128, KI2, tw], BF16, tag="h")
                for m in range(KI2):
                    # Prefetch upcoming x tiles mid-FFN1, far ahead of
                    # their first psum group.  When the next tile is the
                    # short unit tail, also prefetch the one after it.
                    if m == 8 and si + 1 < len(sched):
                        emit_x(si + 1, nc.scalar)
                    if (m == 12 and si + 2 < len(sched)
                            and sched[si + 1][1] <= 128):
                        emit_x(si + 2, nc.scalar)
                    ps = ps1pool.tile([128, tw], F32, tag="ps1")
                    for k in range(KH):
                        nc.tensor.matmul(
                            ps[:],
                            w1s[:, k, m * 128:(m + 1) * 128],
                            xs[:, k, :],
                            start=(k == 0),
                            stop=(k == KH - 1),
                        )
                    nc.scalar.activation(
                        ht[:, m, :],
                        ps[:],
                        mybir.ActivationFunctionType.Gelu,
                        bias=b1s[:, m:m + 1],
                    )
                for m in range(KH):
                    ps = ps2pool.tile([128, tw], F32, tag="ps2")
                    for k in range(KI2):
                        nc.tensor.matmul(
                            ps[:],
                            w2s[:, k, m * 128:(m + 1) * 128],
                            ht[:, k, :],
                            start=(k == 0),
                            stop=(k == KI2 - 1),
                        )
                    # Partial outputs in bf16: halves y traffic and the
                    # final critical-path write; the ~0.2% quantization on
                    # each half partial is far inside the error budget.
                    ot = opool.tile([128, tw], BF16, tag="o")
                    nc.vector.tensor_copy(ot[:], ps[:])
                    if si == len(sched) - 1 and m == KH - 1 and tw >= 256:
                        # Split the very last write across both queues.
                        hw = tw // 2
                        nc.sync.dma_start(
                            yt[m * 128:(m + 1) * 128, off:off + hw],
                            ot[:, 0:hw],
                        )
                        nc.scalar.dma_start(
                            yt[m * 128:(m + 1) * 128, off + hw:off + tw],
                            ot[:, hw:tw],
                        )
                    else:
                        nc.sync.dma_start(
                            yt[m * 128:(m + 1) * 128, off:off + tw], ot[:]
                        )
    _split_waits(nc)
    return nc


def _route(x, gate_w):
    """Host gate: top-2 of 8 logits + softmax over the selected pair."""
    logits = x @ gate_w.T                         # [T, E] f32
    T = logits.shape[0]
    rows = np.arange(T)
    i1 = np.argmax(logits, axis=1)
    v1 = logits[rows, i1]
    masked = logits.copy()
    masked[rows, i1] = -np.inf
    i2 = np.argmax(masked, axis=1)
    v2 = masked[rows, i2]
    # softmax over (v1, v2) with v1 >= v2
    e2 = np.exp(v2 - v1)
    w1 = 1.0 / (1.0 + e2)
    w2 = 1.0 - w1
    return i1, i2, w1.astype(np.float32), w2.astype(np.float32)


def _run(inputs, trace=False):
    hidden_states = np.asarray(inputs["hidden_states"], dtype=np.float32)
    gate_w = np.asarray(inputs["gate_w"], dtype=np.float32)
    W1 = np.asarray(inputs["W1"], dtype=np.float32)
    b1 = np.asarray(inputs["b1"], dtype=np.float32)
    W2 = np.asarray(inputs["W2"], dtype=np.float32)
    b2 = np.asarray(inputs["b2"], dtype=np.float32)

    B, S, _ = hidden_states.shape
    T = B * S
    x = np.ascontiguousarray(hidden_states.reshape(T, H))

    i1, i2, w1, w2 = _route(x, gate_w)
    toks = [np.flatnonzero((i1 == e) | (i2 == e)) for e in range(E)]
    cnts = [len(t) for t in toks]

    # 16 half-expert units (e, half) sorted by token count: A-slots get the
    # 8 heaviest, B-slots the 8 lightest; core i runs units[i] + units[8+i].
    units = sorted(
        [(e, hf) for e in range(E) for hf in range(2)],
        key=lambda u: -cnts[u[0]],
    )
    pad = lambda n: max(128, -(-n // 128) * 128)
    C_A = pad(max(cnts[e] for e, _ in units[:NCORES]))
    C_B = pad(max(cnts[e] for e, _ in units[NCORES:]))

    nc = _build([C_A, C_B])

    xes = {}
    for e in range(E):
        xe = np.zeros((max(C_A, C_B), H), dtype=ml_dtypes.bfloat16)
        xe[: cnts[e]] = x[toks[e]].astype(ml_dtypes.bfloat16)
        xes[e] = np.ascontiguousarray(xe.T)                # [H, Cmax]

    in_maps = []
    for core in range(NCORES):
        m = {}
        for u, C in ((0, C_A), (1, C_B)):
            e, hf = units[u * NCORES + core]
            s = "ab"[u]
            w1h = W1[e][hf * I2:(hf + 1) * I2, :]          # [I2, H]
            w2h = W2[e][:, hf * I2:(hf + 1) * I2]          # [H, I2]
            m[f"x{s}"] = np.ascontiguousarray(xes[e][:, :C])
            m[f"w1{s}"] = np.ascontiguousarray(
                w1h.astype(ml_dtypes.bfloat16).T)          # [H, I2]
            m[f"w2{s}"] = np.ascontiguousarray(
                w2h.astype(ml_dtypes.bfloat16).T)          # [I2, H]
            m[f"b1{s}"] = np.ascontiguousarray(
                b1[e][hf * I2:(hf + 1) * I2].reshape(KI2, 128).T)
        in_maps.append(m)

    res = run_bass_kernel_spmd(
        nc, in_maps, core_ids=list(range(NCORES)), trace=trace
    )

    # Sum each expert's two half-unit partials, add b2, scatter-combine.
    acc = {e: None for e in range(E)}
    for core in range(NCORES):
        for u in range(2):
            e, _ = units[u * NCORES + core]
            ye = res.results[core]["y" + "ab"[u]][:, : cnts[e]].T.astype(
                np.float32)
            acc[e] = ye if acc[e] is None else acc[e] + ye
    out = np.zeros((T, H), dtype=np.float32)
    for e in range(E):
        te = toks[e]
        we = np.where(i1[te] == e, w1[te], w2[te])
        out[te] += we[:, None] * (acc[e] + b2[e])
    return out.reshape(B, S, H), res


def kernel(**inputs):
    out, _ = _run(inputs, trace=False)
    return out

